# revision 14
# baseline (speedup 1.0000x reference)
"""EMA recurrence kernel for Trainium2 (8 NeuronCores, Bass/Tile) — v5.

Computes a_t = w * x_t + (1 - w) * a_{t-1} over inputs [B=32, T=8192, C=128],
initial_state [B, C], weights [C] -> output [B, T, C].

Strategy (v5 = v4 pair decomposition, rebalanced):
  Odd/even split of the recurrence:
      z_i      = x_{2i+1} + c * x_{2i}          (PE: diag(c)/identity matmuls)
      y_{2i+1} = c^2 * y_{2i-1} + z_i           (DVE scan, T/2 elements)
      y_{2i}   = c * y_{2i-1} + x_{2i}          (PE matmuls on shifted y_odd)
  v4 lessons applied:
    - ACT's strided interleaved writes cost +45%: v5 keeps odd/even output
      streams in separate contiguous tiles; the host re-interleaves (layout
      work only).
    - a = w*y scale: odd stream on DVE tensor_scalar (4x fp16 mode), even
      stream folded into the ACT PSUM->SBUF evacuation.
    - PE matmuls grouped per phase across the batch pair (denser bursts,
      same-stationary matmuls adjacent).
  - Batch dim sharded 4-per-core across 8 cores; host supplies [B, C, T]
    fp16, receives odd/even planes fp16 (16-bit IO = HBM floor ~48us/core).
  - Scan decay c^2 stays fp32; scan state is fp32 internally; y0 = a_0/w.
"""

import sys

if "/opt/trn_rl_repo" not in sys.path:
    sys.path.insert(0, "/opt/trn_rl_repo")

import numpy as np

B, T, C = 32, 8192, 128
NCORES = 8
BL = B // NCORES      # batches per core (4)
HALF = T // 2         # in/out DMA granularity (4096 time steps)
HL = HALF // 2        # odd (or even) elements per half (2048)
SPAN = 2048           # time steps per scan chunk
NSP = HALF // SPAN    # chunks per half (2)
L = SPAN // 2         # scan elements per chunk (1024)

_NC_CACHE = None


def build_bass():
    global _NC_CACHE
    if _NC_CACHE is not None:
        return _NC_CACHE

    import concourse.bacc as bacc
    import concourse.mybir as mybir
    import concourse.tile as tile

    f32 = mybir.dt.float32
    f16 = mybir.dt.float16
    AF = mybir.ActivationFunctionType
    ALU = mybir.AluOpType

    nc = bacc.Bacc("TRN2", target_bir_lowering=False, debug=False)
    x = nc.dram_tensor("x", [BL, C, T], f16, kind="ExternalInput").ap()
    cdec2 = nc.dram_tensor("cdec2", [C, L], f32, kind="ExternalInput").ap()
    cdiag = nc.dram_tensor("cdiag", [C, C], f16, kind="ExternalInput").ap()
    wdiag = nc.dram_tensor("wdiag", [C, C], f16, kind="ExternalInput").ap()
    wcdiag = nc.dram_tensor("wcdiag", [C, C], f16, kind="ExternalInput").ap()
    s0T = nc.dram_tensor("s0T", [C, BL], f32, kind="ExternalInput").ap()
    # planes: [b, h, 0=odd/1=even, c, i];  t = h*HALF + 2i + (1 - parity)
    y = nc.dram_tensor("y", [BL, 2, 2, C, HL], f16, kind="ExternalOutput").ap()

    with tile.TileContext(nc) as tc:
        with (
            tc.tile_pool(name="const", bufs=1) as cpool,
            tc.tile_pool(name="xin", bufs=2) as xin_pool,
            tc.tile_pool(name="ysc", bufs=3) as ysc_pool,
            tc.tile_pool(name="yeven", bufs=2) as yeven_pool,
            tc.tile_pool(name="psz", bufs=2, space="PSUM") as psz_pool,
            tc.tile_pool(name="pse", bufs=2, space="PSUM") as pse_pool,
        ):
            wcdiag_t = cpool.tile([C, C], f16, name="wcdiag_t")
            nc.scalar.dma_start(wcdiag_t[:], wcdiag[:])
            wdiag_t = cpool.tile([C, C], f16, name="wdiag_t")
            nc.scalar.dma_start(wdiag_t[:], wdiag[:])
            cdiag_t = cpool.tile([C, C], f16, name="cdiag_t")
            nc.scalar.dma_start(cdiag_t[:], cdiag[:])
            s0T_t = cpool.tile([C, BL], f32, name="s0T_t")
            nc.scalar.dma_start(s0T_t[:], s0T[:])
            cdec2_t = cpool.tile([C, L], f32, name="cdec2_t")
            nc.scalar.dma_start(cdec2_t[:], cdec2[:])

            prev = {}
            for pair in range(BL // 2):
                bs = (2 * pair, 2 * pair + 1)
                for h in range(2):
                    xin, yeven = {}, {}
                    for b in bs:
                        xt = xin_pool.tile(
                            [C, HALF], f16, name=f"xin{b}_{h}", tag=f"xin{b % 2}"
                        )
                        for k2 in range(NSP):
                            nc.sync.dma_start(
                                xt[:, k2 * SPAN : (k2 + 1) * SPAN],
                                x[b][
                                    :,
                                    h * HALF + k2 * SPAN : h * HALF + (k2 + 1) * SPAN,
                                ],
                            )
                        xin[b] = xt
                        yeven[b] = yeven_pool.tile(
                            [C, HL], f16, name=f"ye{b}_{h}", tag=f"ye{b % 2}"
                        )
                    for k in range(NSP):
                        xv = {
                            b: xin[b].rearrange(
                                "c (s i two) -> c s i two", two=2, i=512
                            )
                            for b in bs
                        }
                        subs = (2 * k, 2 * k + 1)
                        # --- z phase: z = c*x_even + x_odd (both batches) ---
                        psz = {
                            b: psz_pool.tile([C, 2, 512], f32, name="psz", tag="psz")
                            for b in bs
                        }
                        for b in bs:
                            for si, s in enumerate(subs):
                                nc.tensor.matmul(
                                    psz[b][:, si, :], wcdiag_t[:], xv[b][:, s, :, 0],
                                    start=True, stop=False,
                                )
                        for b in bs:
                            for si, s in enumerate(subs):
                                nc.tensor.matmul(
                                    psz[b][:, si, :], wdiag_t[:], xv[b][:, s, :, 1],
                                    start=False, stop=True,
                                )
                        # --- scans ---
                        ysc = {}
                        for b in bs:
                            yt = ysc_pool.tile(
                                [C, L + 1], f16, name="ysc", tag=f"ysc{b % 2}"
                            )
                            first = h == 0 and k == 0
                            init = (
                                s0T_t[:, b : b + 1]
                                if first
                                else prev[b][:, L : L + 1]
                            )
                            nc.gpsimd.tensor_copy(yt[:, 0:1], init)
                            nc.vector.tensor_tensor_scan(
                                yt[:, 1 : L + 1],
                                cdec2_t[:],
                                psz[b].rearrange("c s i -> c (s i)"),
                                init,
                                op0=ALU.mult,
                                op1=ALU.add,
                            )
                            prev[b] = ysc[b] = yt
                        # --- reconstruct: a_even = c*a_shift + w*x_even ---
                        pse = {
                            b: pse_pool.tile([C, 2, 512], f32, name="pse", tag="pse")
                            for b in bs
                        }
                        for b in bs:
                            for si in range(2):
                                nc.tensor.matmul(
                                    pse[b][:, si, :], cdiag_t[:],
                                    ysc[b][:, si * 512 : si * 512 + 512],
                                    start=True, stop=False,
                                )
                        for b in bs:
                            for si, s in enumerate(subs):
                                nc.tensor.matmul(
                                    pse[b][:, si, :], wdiag_t[:], xv[b][:, s, :, 0],
                                    start=False, stop=True,
                                )
                        for b in bs:
                            # odd plane straight from the scan output (SWDGE:
                            # keeps dispatch off the ACT HWDGE ring)
                            nc.gpsimd.dma_start(
                                y[b][h][0][:, k * L : (k + 1) * L],
                                ysc[b][:, 1 : L + 1],
                            )
                            nc.scalar.activation(
                                yeven[b][:, k * L : (k + 1) * L],
                                pse[b].rearrange("c s i -> c (s i)"),
                                AF.Copy,
                            )
                            nc.gpsimd.dma_start(
                                y[b][h][1][:, k * L : (k + 1) * L],
                                yeven[b][:, k * L : (k + 1) * L],
                            )

    nc.compile()
    _NC_CACHE = nc
    return nc


def _in_maps(inputs, initial_state, weights):
    x = np.asarray(inputs, dtype=np.float32)
    s0 = np.asarray(initial_state, dtype=np.float64)
    w = np.clip(np.asarray(weights, dtype=np.float64), 0.0, 1.0)
    c = 1.0 - w

    xT = np.ascontiguousarray(x.transpose(0, 2, 1)).astype(np.float16)
    cdec2 = np.ascontiguousarray(
        np.repeat((c * c).astype(np.float32)[:, None], L, axis=1)
    )
    cdiag = np.ascontiguousarray(np.diag(c).astype(np.float16))
    wdiag = np.ascontiguousarray(np.diag(w).astype(np.float16))
    wcdiag = np.ascontiguousarray(np.diag(w * c).astype(np.float16))
    y0 = s0.astype(np.float32)  # [B, C] initial state directly (a-space)

    maps = []
    for i in range(NCORES):
        maps.append(
            {
                "x": np.ascontiguousarray(xT[i * BL : (i + 1) * BL]),
                "cdec2": cdec2,
                "cdiag": cdiag,
                "wdiag": wdiag,
                "wcdiag": wcdiag,
                "s0T": np.ascontiguousarray(y0[i * BL : (i + 1) * BL].T),
            }
        )
    return maps


def _unpermute(y_perm):
    # y_perm: [BL, 2(h), 2(0=odd,1=even), C, HL] fp16 -> [BL, T, C] fp32
    # t = h*HALF + 2i + (1 if parity==0 else 0); so ordering (even, odd)
    # along the last axis of pairs -> reverse parity axis then interleave.
    yp = y_perm[:, :, ::-1]                # [BL, h, (even,odd), C, HL]
    yp = yp.transpose(0, 1, 4, 2, 3)       # [BL, h, i, (even,odd), C]
    return yp.reshape(BL, T, C).astype(np.float32)


def _ensure_ntff_hook():
    """Shim antenv.axon_hooks (absent in this image) so trace=True works."""
    import types

    import antenv

    if not hasattr(antenv, "axon_hooks"):
        mod = types.ModuleType("antenv.axon_hooks")
        holder = [None]
        mod.set_axon_ntff_profile_hook = lambda h: holder.__setitem__(0, h)
        mod.get_axon_ntff_profile_hook = lambda: holder[0]
        sys.modules["antenv.axon_hooks"] = mod
        antenv.axon_hooks = mod
    from antenv.axon_hooks import (
        get_axon_ntff_profile_hook,
        set_axon_ntff_profile_hook,
    )

    if get_axon_ntff_profile_hook() is None:
        from trn_agent_boot.trn_boot import _ntff_profile_via_ctypes

        set_axon_ntff_profile_hook(
            _ntff_profile_via_ctypes("/opt/axon/libaxon_pjrt.so")
        )


def run(inputs, initial_state, weights, trace=False, **kw):
    from concourse import bass_utils

    if trace:
        _ensure_ntff_hook()
    nc = build_bass()
    maps = _in_maps(inputs, initial_state, weights)
    res = bass_utils.run_bass_kernel_spmd(
        nc, maps, core_ids=list(range(NCORES)), trace=trace, **kw
    )
    out = np.concatenate([_unpermute(r["y"]) for r in res.results], axis=0)
    return out, res


def kernel(inputs, initial_state, weights):
    out, _ = run(inputs, initial_state, weights)
    return out


# revision 15
# speedup vs baseline: 1.0380x; 1.0380x over previous
"""EMA recurrence kernel for Trainium2 (8 NeuronCores, Bass/Tile) — v5.

Computes a_t = w * x_t + (1 - w) * a_{t-1} over inputs [B=32, T=8192, C=128],
initial_state [B, C], weights [C] -> output [B, T, C].

Strategy (v5 = v4 pair decomposition, rebalanced):
  Odd/even split of the recurrence:
      z_i      = x_{2i+1} + c * x_{2i}          (PE: diag(c)/identity matmuls)
      y_{2i+1} = c^2 * y_{2i-1} + z_i           (DVE scan, T/2 elements)
      y_{2i}   = c * y_{2i-1} + x_{2i}          (PE matmuls on shifted y_odd)
  v4 lessons applied:
    - ACT's strided interleaved writes cost +45%: v5 keeps odd/even output
      streams in separate contiguous tiles; the host re-interleaves (layout
      work only).
    - a = w*y scale: odd stream on DVE tensor_scalar (4x fp16 mode), even
      stream folded into the ACT PSUM->SBUF evacuation.
    - PE matmuls grouped per phase across the batch pair (denser bursts,
      same-stationary matmuls adjacent).
  - Batch dim sharded 4-per-core across 8 cores; host supplies [B, C, T]
    fp16, receives odd/even planes fp16 (16-bit IO = HBM floor ~48us/core).
  - Scan decay c^2 stays fp32; scan state is fp32 internally; y0 = a_0/w.
"""

import sys

if "/opt/trn_rl_repo" not in sys.path:
    sys.path.insert(0, "/opt/trn_rl_repo")

import numpy as np

B, T, C = 32, 8192, 128
NCORES = 8
BL = B // NCORES      # batches per core (4)
HALF = T // 2         # in/out DMA granularity (4096 time steps)
HL = HALF // 2        # odd (or even) elements per half (2048)
SPAN = 2048           # time steps per scan chunk
NSP = HALF // SPAN    # chunks per half (2)
L = SPAN // 2         # scan elements per chunk (1024)

_NC_CACHE = None


def build_bass():
    global _NC_CACHE
    if _NC_CACHE is not None:
        return _NC_CACHE

    import concourse.bacc as bacc
    import concourse.mybir as mybir
    import concourse.tile as tile

    f32 = mybir.dt.float32
    f16 = mybir.dt.float16
    AF = mybir.ActivationFunctionType
    ALU = mybir.AluOpType

    nc = bacc.Bacc("TRN2", target_bir_lowering=False, debug=False)
    x = nc.dram_tensor("x", [BL, C, T], f16, kind="ExternalInput").ap()
    cdec2 = nc.dram_tensor("cdec2", [C, L], f32, kind="ExternalInput").ap()
    cdiag = nc.dram_tensor("cdiag", [C, C], f16, kind="ExternalInput").ap()
    wdiag = nc.dram_tensor("wdiag", [C, C], f16, kind="ExternalInput").ap()
    wcdiag = nc.dram_tensor("wcdiag", [C, C], f16, kind="ExternalInput").ap()
    s0T = nc.dram_tensor("s0T", [C, BL], f32, kind="ExternalInput").ap()
    # planes: [b, h, 0=odd/1=even, c, i];  t = h*HALF + 2i + (1 - parity)
    y = nc.dram_tensor("y", [BL, 2, 2, C, HL], f16, kind="ExternalOutput").ap()

    with tile.TileContext(nc) as tc:
        with (
            tc.tile_pool(name="const", bufs=1) as cpool,
            tc.tile_pool(name="xin", bufs=2) as xin_pool,
            tc.tile_pool(name="ysc", bufs=3) as ysc_pool,
            tc.tile_pool(name="yeven", bufs=2) as yeven_pool,
            tc.tile_pool(name="psz", bufs=2, space="PSUM") as psz_pool,
            tc.tile_pool(name="pse", bufs=2, space="PSUM") as pse_pool,
        ):
            wcdiag_t = cpool.tile([C, C], f16, name="wcdiag_t")
            nc.scalar.dma_start(wcdiag_t[:], wcdiag[:])
            wdiag_t = cpool.tile([C, C], f16, name="wdiag_t")
            nc.scalar.dma_start(wdiag_t[:], wdiag[:])
            cdiag_t = cpool.tile([C, C], f16, name="cdiag_t")
            nc.scalar.dma_start(cdiag_t[:], cdiag[:])
            s0T_t = cpool.tile([C, BL], f32, name="s0T_t")
            nc.scalar.dma_start(s0T_t[:], s0T[:])
            cdec2_t = cpool.tile([C, L], f32, name="cdec2_t")
            nc.scalar.dma_start(cdec2_t[:], cdec2[:])

            prev = {}
            for pair in range(BL // 2):
                bs = (2 * pair, 2 * pair + 1)
                for h in range(2):
                    xin, yeven = {}, {}
                    for b in bs:
                        xt = xin_pool.tile(
                            [C, HALF], f16, name=f"xin{b}_{h}", tag=f"xin{b % 2}"
                        )
                        for k2 in range(NSP):
                            nc.sync.dma_start(
                                xt[:, k2 * SPAN : (k2 + 1) * SPAN],
                                x[b][
                                    :,
                                    h * HALF + k2 * SPAN : h * HALF + (k2 + 1) * SPAN,
                                ],
                            )
                        xin[b] = xt
                        yeven[b] = yeven_pool.tile(
                            [C, HL], f16, name=f"ye{b}_{h}", tag=f"ye{b % 2}"
                        )
                    for k in range(NSP):
                        xv = {
                            b: xin[b].rearrange(
                                "c (s i two) -> c s i two", two=2, i=512
                            )
                            for b in bs
                        }
                        subs = (2 * k, 2 * k + 1)
                        # --- z phase: z = c*x_even + x_odd (both batches) ---
                        psz = {
                            b: psz_pool.tile([C, 2, 512], f32, name="psz", tag="psz")
                            for b in bs
                        }
                        for b in bs:
                            for si, s in enumerate(subs):
                                nc.tensor.matmul(
                                    psz[b][:, si, :], wcdiag_t[:], xv[b][:, s, :, 0],
                                    start=True, stop=False,
                                )
                        for b in bs:
                            for si, s in enumerate(subs):
                                nc.tensor.matmul(
                                    psz[b][:, si, :], wdiag_t[:], xv[b][:, s, :, 1],
                                    start=False, stop=True,
                                )
                        # --- scans ---
                        ysc = {}
                        for b in bs:
                            yt = ysc_pool.tile(
                                [C, L + 1], f16, name="ysc", tag=f"ysc{b % 2}"
                            )
                            first = h == 0 and k == 0
                            init = (
                                s0T_t[:, b : b + 1]
                                if first
                                else prev[b][:, L : L + 1]
                            )
                            nc.gpsimd.tensor_copy(yt[:, 0:1], init)
                            nc.vector.tensor_tensor_scan(
                                yt[:, 1 : L + 1],
                                cdec2_t[:],
                                psz[b].rearrange("c s i -> c (s i)"),
                                init,
                                op0=ALU.mult,
                                op1=ALU.add,
                            )
                            prev[b] = ysc[b] = yt
                        # --- reconstruct: a_even = c*a_shift + w*x_even ---
                        pse = {
                            b: pse_pool.tile([C, 2, 512], f32, name="pse", tag="pse")
                            for b in bs
                        }
                        for b in bs:
                            for si in range(2):
                                nc.tensor.matmul(
                                    pse[b][:, si, :], cdiag_t[:],
                                    ysc[b][:, si * 512 : si * 512 + 512],
                                    start=True, stop=False,
                                )
                        for b in bs:
                            for si, s in enumerate(subs):
                                nc.tensor.matmul(
                                    pse[b][:, si, :], wdiag_t[:], xv[b][:, s, :, 0],
                                    start=False, stop=True,
                                )
                        for b in bs:
                            # odd plane straight from the scan output
                            nc.scalar.dma_start(
                                y[b][h][0][:, k * L : (k + 1) * L],
                                ysc[b][:, 1 : L + 1],
                            )
                            nc.scalar.activation(
                                yeven[b][:, k * L : (k + 1) * L],
                                pse[b].rearrange("c s i -> c (s i)"),
                                AF.Copy,
                            )
                    for b in bs:
                        nc.scalar.dma_start(y[b][h][1], yeven[b][:])

    nc.compile()
    _NC_CACHE = nc
    return nc


def _in_maps(inputs, initial_state, weights):
    x = np.asarray(inputs, dtype=np.float32)
    s0 = np.asarray(initial_state, dtype=np.float64)
    w = np.clip(np.asarray(weights, dtype=np.float64), 0.0, 1.0)
    c = 1.0 - w

    xT = np.ascontiguousarray(x.transpose(0, 2, 1)).astype(np.float16)
    cdec2 = np.ascontiguousarray(
        np.repeat((c * c).astype(np.float32)[:, None], L, axis=1)
    )
    cdiag = np.ascontiguousarray(np.diag(c).astype(np.float16))
    wdiag = np.ascontiguousarray(np.diag(w).astype(np.float16))
    wcdiag = np.ascontiguousarray(np.diag(w * c).astype(np.float16))
    y0 = s0.astype(np.float32)  # [B, C] initial state directly (a-space)

    maps = []
    for i in range(NCORES):
        maps.append(
            {
                "x": np.ascontiguousarray(xT[i * BL : (i + 1) * BL]),
                "cdec2": cdec2,
                "cdiag": cdiag,
                "wdiag": wdiag,
                "wcdiag": wcdiag,
                "s0T": np.ascontiguousarray(y0[i * BL : (i + 1) * BL].T),
            }
        )
    return maps


def _unpermute(y_perm):
    # y_perm: [BL, 2(h), 2(0=odd,1=even), C, HL] fp16 -> [BL, T, C] fp32
    # t = h*HALF + 2i + (1 if parity==0 else 0); so ordering (even, odd)
    # along the last axis of pairs -> reverse parity axis then interleave.
    yp = y_perm[:, :, ::-1]                # [BL, h, (even,odd), C, HL]
    yp = yp.transpose(0, 1, 4, 2, 3)       # [BL, h, i, (even,odd), C]
    return yp.reshape(BL, T, C).astype(np.float32)


def _ensure_ntff_hook():
    """Shim antenv.axon_hooks (absent in this image) so trace=True works."""
    import types

    import antenv

    if not hasattr(antenv, "axon_hooks"):
        mod = types.ModuleType("antenv.axon_hooks")
        holder = [None]
        mod.set_axon_ntff_profile_hook = lambda h: holder.__setitem__(0, h)
        mod.get_axon_ntff_profile_hook = lambda: holder[0]
        sys.modules["antenv.axon_hooks"] = mod
        antenv.axon_hooks = mod
    from antenv.axon_hooks import (
        get_axon_ntff_profile_hook,
        set_axon_ntff_profile_hook,
    )

    if get_axon_ntff_profile_hook() is None:
        from trn_agent_boot.trn_boot import _ntff_profile_via_ctypes

        set_axon_ntff_profile_hook(
            _ntff_profile_via_ctypes("/opt/axon/libaxon_pjrt.so")
        )


def run(inputs, initial_state, weights, trace=False, **kw):
    from concourse import bass_utils

    if trace:
        _ensure_ntff_hook()
    nc = build_bass()
    maps = _in_maps(inputs, initial_state, weights)
    res = bass_utils.run_bass_kernel_spmd(
        nc, maps, core_ids=list(range(NCORES)), trace=trace, **kw
    )
    out = np.concatenate([_unpermute(r["y"]) for r in res.results], axis=0)
    return out, res


def kernel(inputs, initial_state, weights):
    out, _ = run(inputs, initial_state, weights)
    return out


# revision 16
# speedup vs baseline: 1.0438x; 1.0056x over previous
"""EMA recurrence kernel for Trainium2 (8 NeuronCores, Bass/Tile) — v5.

Computes a_t = w * x_t + (1 - w) * a_{t-1} over inputs [B=32, T=8192, C=128],
initial_state [B, C], weights [C] -> output [B, T, C].

Strategy (v5 = v4 pair decomposition, rebalanced):
  Odd/even split of the recurrence:
      z_i      = x_{2i+1} + c * x_{2i}          (PE: diag(c)/identity matmuls)
      y_{2i+1} = c^2 * y_{2i-1} + z_i           (DVE scan, T/2 elements)
      y_{2i}   = c * y_{2i-1} + x_{2i}          (PE matmuls on shifted y_odd)
  v4 lessons applied:
    - ACT's strided interleaved writes cost +45%: v5 keeps odd/even output
      streams in separate contiguous tiles; the host re-interleaves (layout
      work only).
    - a = w*y scale: odd stream on DVE tensor_scalar (4x fp16 mode), even
      stream folded into the ACT PSUM->SBUF evacuation.
    - PE matmuls grouped per phase across the batch pair (denser bursts,
      same-stationary matmuls adjacent).
  - Batch dim sharded 4-per-core across 8 cores; host supplies [B, C, T]
    fp16, receives odd/even planes fp16 (16-bit IO = HBM floor ~48us/core).
  - Scan decay c^2 stays fp32; scan state is fp32 internally; y0 = a_0/w.
"""

import sys

if "/opt/trn_rl_repo" not in sys.path:
    sys.path.insert(0, "/opt/trn_rl_repo")

import numpy as np

B, T, C = 32, 8192, 128
NCORES = 8
BL = B // NCORES      # batches per core (4)
HALF = T // 2         # in/out DMA granularity (4096 time steps)
HL = HALF // 2        # odd (or even) elements per half (2048)
SPAN = 2048           # time steps per scan chunk
NSP = HALF // SPAN    # chunks per half (2)
L = SPAN // 2         # scan elements per chunk (1024)

_NC_CACHE = None


def build_bass():
    global _NC_CACHE
    if _NC_CACHE is not None:
        return _NC_CACHE

    import concourse.bacc as bacc
    import concourse.mybir as mybir
    import concourse.tile as tile

    f32 = mybir.dt.float32
    f16 = mybir.dt.float16
    AF = mybir.ActivationFunctionType
    ALU = mybir.AluOpType

    nc = bacc.Bacc("TRN2", target_bir_lowering=False, debug=False)
    x = nc.dram_tensor("x", [BL, C, T], f16, kind="ExternalInput").ap()
    cdec2 = nc.dram_tensor("cdec2", [C, L], f32, kind="ExternalInput").ap()
    cdiag = nc.dram_tensor("cdiag", [C, C], f16, kind="ExternalInput").ap()
    wdiag = nc.dram_tensor("wdiag", [C, C], f16, kind="ExternalInput").ap()
    wcdiag = nc.dram_tensor("wcdiag", [C, C], f16, kind="ExternalInput").ap()
    s0T = nc.dram_tensor("s0T", [C, BL], f32, kind="ExternalInput").ap()
    # planes: [b, h, 0=odd/1=even, c, i];  t = h*HALF + 2i + (1 - parity)
    y = nc.dram_tensor("y", [BL, 2, 2, C, HL], f16, kind="ExternalOutput").ap()

    with tile.TileContext(nc) as tc:
        with (
            tc.tile_pool(name="const", bufs=1) as cpool,
            tc.tile_pool(name="xin", bufs=2) as xin_pool,
            tc.tile_pool(name="ysc", bufs=3) as ysc_pool,
            tc.tile_pool(name="yeven", bufs=2) as yeven_pool,
            tc.tile_pool(name="psz", bufs=4, space="PSUM") as psz_pool,
            tc.tile_pool(name="pse", bufs=2, space="PSUM") as pse_pool,
        ):
            wcdiag_t = cpool.tile([C, C], f16, name="wcdiag_t")
            nc.scalar.dma_start(wcdiag_t[:], wcdiag[:])
            wdiag_t = cpool.tile([C, C], f16, name="wdiag_t")
            nc.scalar.dma_start(wdiag_t[:], wdiag[:])
            cdiag_t = cpool.tile([C, C], f16, name="cdiag_t")
            nc.scalar.dma_start(cdiag_t[:], cdiag[:])
            s0T_t = cpool.tile([C, BL], f32, name="s0T_t")
            nc.scalar.dma_start(s0T_t[:], s0T[:])
            cdec2_t = cpool.tile([C, L], f32, name="cdec2_t")
            nc.scalar.dma_start(cdec2_t[:], cdec2[:])

            prev = {}
            for pair in range(BL // 2):
                bs = (2 * pair, 2 * pair + 1)
                for h in range(2):
                    xin, yeven = {}, {}
                    for b in bs:
                        xt = xin_pool.tile(
                            [C, HALF], f16, name=f"xin{b}_{h}", tag=f"xin{b % 2}"
                        )
                        for k2 in range(NSP):
                            nc.sync.dma_start(
                                xt[:, k2 * SPAN : (k2 + 1) * SPAN],
                                x[b][
                                    :,
                                    h * HALF + k2 * SPAN : h * HALF + (k2 + 1) * SPAN,
                                ],
                            )
                        xin[b] = xt
                        yeven[b] = yeven_pool.tile(
                            [C, HL], f16, name=f"ye{b}_{h}", tag=f"ye{b % 2}"
                        )
                    for k in range(NSP):
                        xv = {
                            b: xin[b].rearrange(
                                "c (s i two) -> c s i two", two=2, i=512
                            )
                            for b in bs
                        }
                        subs = (2 * k, 2 * k + 1)
                        # --- z + scans at 512 granularity: single-bank psz
                        # tiles (prefetch depth 4) keep PE ahead of DVE ---
                        ysc = {}
                        for b in bs:
                            yt = ysc_pool.tile(
                                [C, L + 1], f16, name="ysc", tag=f"ysc{b % 2}"
                            )
                            first = h == 0 and k == 0
                            init0 = (
                                s0T_t[:, b : b + 1]
                                if first
                                else prev[b][:, L : L + 1]
                            )
                            nc.gpsimd.tensor_copy(yt[:, 0:1], init0)
                            for si, s in enumerate(subs):
                                psz = psz_pool.tile(
                                    [C, 512], f32, name="psz", tag="psz"
                                )
                                nc.tensor.matmul(
                                    psz[:], wcdiag_t[:], xv[b][:, s, :, 0],
                                    start=True, stop=False,
                                )
                                nc.tensor.matmul(
                                    psz[:], wdiag_t[:], xv[b][:, s, :, 1],
                                    start=False, stop=True,
                                )
                                init = init0 if si == 0 else yt[:, 512:513]
                                nc.vector.tensor_tensor_scan(
                                    yt[:, 1 + si * 512 : 1 + si * 512 + 512],
                                    cdec2_t[:, :512],
                                    psz[:],
                                    init,
                                    op0=ALU.mult,
                                    op1=ALU.add,
                                )
                            prev[b] = ysc[b] = yt
                        # --- reconstruct: a_even = c*a_shift + w*x_even ---
                        pse = {
                            b: pse_pool.tile([C, 2, 512], f32, name="pse", tag="pse")
                            for b in bs
                        }
                        for b in bs:
                            for si in range(2):
                                nc.tensor.matmul(
                                    pse[b][:, si, :], cdiag_t[:],
                                    ysc[b][:, si * 512 : si * 512 + 512],
                                    start=True, stop=False,
                                )
                        for b in bs:
                            for si, s in enumerate(subs):
                                nc.tensor.matmul(
                                    pse[b][:, si, :], wdiag_t[:], xv[b][:, s, :, 0],
                                    start=False, stop=True,
                                )
                        for b in bs:
                            # odd plane straight from the scan output
                            nc.scalar.dma_start(
                                y[b][h][0][:, k * L : (k + 1) * L],
                                ysc[b][:, 1 : L + 1],
                            )
                            nc.scalar.activation(
                                yeven[b][:, k * L : (k + 1) * L],
                                pse[b].rearrange("c s i -> c (s i)"),
                                AF.Copy,
                            )
                    for b in bs:
                        nc.scalar.dma_start(y[b][h][1], yeven[b][:])

    nc.compile()
    _NC_CACHE = nc
    return nc


def _in_maps(inputs, initial_state, weights):
    x = np.asarray(inputs, dtype=np.float32)
    s0 = np.asarray(initial_state, dtype=np.float64)
    w = np.clip(np.asarray(weights, dtype=np.float64), 0.0, 1.0)
    c = 1.0 - w

    xT = np.ascontiguousarray(x.transpose(0, 2, 1)).astype(np.float16)
    cdec2 = np.ascontiguousarray(
        np.repeat((c * c).astype(np.float32)[:, None], L, axis=1)
    )
    cdiag = np.ascontiguousarray(np.diag(c).astype(np.float16))
    wdiag = np.ascontiguousarray(np.diag(w).astype(np.float16))
    wcdiag = np.ascontiguousarray(np.diag(w * c).astype(np.float16))
    y0 = s0.astype(np.float32)  # [B, C] initial state directly (a-space)

    maps = []
    for i in range(NCORES):
        maps.append(
            {
                "x": np.ascontiguousarray(xT[i * BL : (i + 1) * BL]),
                "cdec2": cdec2,
                "cdiag": cdiag,
                "wdiag": wdiag,
                "wcdiag": wcdiag,
                "s0T": np.ascontiguousarray(y0[i * BL : (i + 1) * BL].T),
            }
        )
    return maps


def _unpermute(y_perm):
    # y_perm: [BL, 2(h), 2(0=odd,1=even), C, HL] fp16 -> [BL, T, C] fp32
    # t = h*HALF + 2i + (1 if parity==0 else 0); so ordering (even, odd)
    # along the last axis of pairs -> reverse parity axis then interleave.
    yp = y_perm[:, :, ::-1]                # [BL, h, (even,odd), C, HL]
    yp = yp.transpose(0, 1, 4, 2, 3)       # [BL, h, i, (even,odd), C]
    return yp.reshape(BL, T, C).astype(np.float32)


def _ensure_ntff_hook():
    """Shim antenv.axon_hooks (absent in this image) so trace=True works."""
    import types

    import antenv

    if not hasattr(antenv, "axon_hooks"):
        mod = types.ModuleType("antenv.axon_hooks")
        holder = [None]
        mod.set_axon_ntff_profile_hook = lambda h: holder.__setitem__(0, h)
        mod.get_axon_ntff_profile_hook = lambda: holder[0]
        sys.modules["antenv.axon_hooks"] = mod
        antenv.axon_hooks = mod
    from antenv.axon_hooks import (
        get_axon_ntff_profile_hook,
        set_axon_ntff_profile_hook,
    )

    if get_axon_ntff_profile_hook() is None:
        from trn_agent_boot.trn_boot import _ntff_profile_via_ctypes

        set_axon_ntff_profile_hook(
            _ntff_profile_via_ctypes("/opt/axon/libaxon_pjrt.so")
        )


def run(inputs, initial_state, weights, trace=False, **kw):
    from concourse import bass_utils

    if trace:
        _ensure_ntff_hook()
    nc = build_bass()
    maps = _in_maps(inputs, initial_state, weights)
    res = bass_utils.run_bass_kernel_spmd(
        nc, maps, core_ids=list(range(NCORES)), trace=trace, **kw
    )
    out = np.concatenate([_unpermute(r["y"]) for r in res.results], axis=0)
    return out, res


def kernel(inputs, initial_state, weights):
    out, _ = run(inputs, initial_state, weights)
    return out
